# revision 9
# baseline (speedup 1.0000x reference)
"""CRF NLL kernel for Trainium2 (8 NeuronCores, batch-parallel).

Math: the CRF forward recursion
    part_t[j] = logsumexp_i(part_{t-1}[i] + trans[i,j]) + feat[t,j]
is run in the exponential domain:
    p_t[j,b] = (sum_i p_{t-1}[i,b] * E[i,j]) * F_t[j,b]
with E = exp(trans) and F_t = exp(feat_t - lognorm_t) the *normalized*
emission weights (per-(t,b) log-normalizers are folded back in on the
host). Normalizing F keeps p_t's magnitude drift bounded within fp32
range over all 256 steps, so the device scan needs no rescaling, no
max-subtraction, and no mask handling (rows past their length are
garbage but never read — the host gathers each row's state at t=len-1
from the stored trajectory).

Per core: 8 of the 64 sequences; state kept T-major (64 tag partitions
x 8 batch cols) so each step is one weight-stationary PE matmul
(lhsT=E) plus one DVE multiply PSUM*F -> SBUF written straight into
the trajectory buffer.
"""

import sys

sys.path.insert(0, "/opt/trn_rl_repo")

import numpy as np

B, S, TAG = 64, 256, 64
START, END = TAG - 2, TAG - 1
NCORES = 8
BLOC = B // NCORES  # 8 sequences per core

_compiled = {}


def _build_nc():
    import concourse.bass as bass
    import concourse.bacc as bacc
    import concourse.mybir as mybir
    from concourse import tile

    f32 = mybir.dt.float32
    nc = bacc.Bacc(
        "TRN2", target_bir_lowering=False, debug=False, num_devices=NCORES
    )

    ft_d = nc.dram_tensor("ft", [TAG, S * BLOC], f32, kind="ExternalInput")
    e_d = nc.dram_tensor("e", [TAG, TAG], f32, kind="ExternalInput")
    out_d = nc.dram_tensor("out", [TAG, S * BLOC], f32, kind="ExternalOutput")

    with tile.TileContext(nc) as tc:
        with (
            tc.tile_pool(name="pool", bufs=1) as pool,
            tc.tile_pool(name="stage", bufs=4) as stage,
            tc.tile_pool(name="psum", bufs=8, space=bass.MemorySpace.PSUM) as psum,
        ):
            e_t = pool.tile([TAG, TAG], f32)
            ft_t = pool.tile([TAG, S * BLOC], f32)
            snap = pool.tile([TAG, S * BLOC], f32)

            # All DRAM loads are staged through a DVE copy: this walrus build
            # fits only ONE sync-wait per instruction, so every consumer must
            # depend on a single semaphore (DVE's); same-engine deps are free.
            e_stage = stage.tile([TAG, TAG], f32, tag="est")
            nc.sync.dma_start(e_stage[:], e_d[:])
            nc.vector.tensor_copy(e_t[:], e_stage[:])
            # chunk the big load so step 0 can start early
            NCH = 8
            ch = S * BLOC // NCH
            for k in range(NCH):
                stg = stage.tile([TAG, ch], f32, tag="ftstage")
                nc.sync.dma_start(stg[:], ft_d[:, k * ch : (k + 1) * ch])
                nc.vector.tensor_copy(ft_t[:, k * ch : (k + 1) * ch], stg[:])

            # init: p0 = F0 * exp(trans[START,:]) — estart pre-folded on host
            nc.vector.tensor_copy(snap[:, 0:BLOC], ft_t[:, 0:BLOC])

            for t in range(1, S):
                ps = psum.tile([TAG, BLOC], f32)
                nc.tensor.matmul(
                    ps[:], e_t[:], snap[:, (t - 1) * BLOC : t * BLOC]
                )
                nc.vector.tensor_mul(
                    snap[:, t * BLOC : (t + 1) * BLOC],
                    ps[:],
                    ft_t[:, t * BLOC : (t + 1) * BLOC],
                )

            for k in range(NCH):
                nc.gpsimd.dma_start(out_d[:, k * ch : (k + 1) * ch], snap[:, k * ch : (k + 1) * ch])

    nc.compile()
    return nc


def _get_nc():
    if "nc" not in _compiled:
        _compiled["nc"] = _build_nc()
    return _compiled["nc"]


def _run_device(in_maps, trace=False):
    from concourse.bass_utils import run_bass_kernel_spmd

    nc = _get_nc()
    return run_bass_kernel_spmd(nc, in_maps, list(range(NCORES)), trace=trace)


def _logsumexp(x, axis=-1):
    m = np.max(x, axis=axis, keepdims=True)
    return np.squeeze(m, axis) + np.log(np.sum(np.exp(x - m), axis=axis))


def prepare_inputs(feats, transitions):
    """Host-side prep shared by kernel() and test harnesses."""
    feats64 = feats.astype(np.float64)
    lognorm = _logsumexp(feats64, axis=2)  # (B,S)
    fnorm = np.exp(feats64 - lognorm[:, :, None]).astype(np.float32)  # (B,S,T)
    tr = transitions.astype(np.float64)
    e_mat = np.ascontiguousarray(np.exp(tr).astype(np.float32))  # (T,T) rows=i
    es = np.exp(tr[START, :]).astype(np.float32)  # (T,)
    in_maps = []
    for c in range(NCORES):
        fc = fnorm[c * BLOC : (c + 1) * BLOC]  # (8,S,T)
        ftc = np.ascontiguousarray(fc.transpose(2, 1, 0).reshape(TAG, S * BLOC))
        ftc[:, :BLOC] *= es[:, None]  # fold start transitions into F_0
        in_maps.append({"ft": ftc, "e": e_mat})
    return in_maps, lognorm


def finish(results, lognorm, feats, mask, tags, transitions):
    """Gather per-length states, add back normalizers, compute NLL."""
    mask = np.asarray(mask).astype(bool)
    tags = np.asarray(tags).astype(np.int64)
    tr = np.asarray(transitions).astype(np.float64)
    lengths = mask.sum(axis=1).astype(np.int64)

    fwd = 0.0
    for b in range(B):
        c, bl = b // BLOC, b % BLOC
        tb = int(lengths[b]) - 1
        pvec = results[c]["out"][:, tb * BLOC + bl].astype(np.float64)
        with np.errstate(divide="ignore"):
            part = np.log(pvec) + lognorm[b, : tb + 1].sum()
        fwd += _logsumexp(part + tr[:, END])

    feats64 = np.asarray(feats).astype(np.float64)
    prev = np.concatenate(
        [np.full((B, 1), START, dtype=np.int64), tags[:, :-1]], axis=1
    )
    emit = np.take_along_axis(feats64, tags[:, :, None], axis=2)[:, :, 0]
    trans_sc = tr[prev, tags]
    tg = np.where(mask, emit + trans_sc, 0.0).sum()
    end_ids = tags[np.arange(B), lengths - 1]
    gold = tg + tr[end_ids, END].sum()

    return np.float32(fwd - gold)


def kernel(feats, mask, tags, transitions):
    feats = np.asarray(feats, dtype=np.float32)
    transitions = np.asarray(transitions, dtype=np.float32)
    in_maps, lognorm = prepare_inputs(feats, transitions)
    res = _run_device(in_maps).results
    return finish(res, lognorm, feats, mask, tags, transitions)
